# revision 1
# baseline (speedup 1.0000x reference)
"""Sparse avg-pool (segment mean) for Trainium2, 8 NeuronCores — TensorEngine version.

Range-shard coarse ids across cores (core k owns ids [k*31360, (k+1)*31360)),
so no collective is needed.  On each core the segment-sum runs on the
TensorEngine: the host sorts the core's rows by local id and buckets them into
245 windows of 128 consecutive ids, padding each window to `cap` tiles of 128
tokens.  For each 128-token tile the DVE builds a one-hot [token, seg] matrix
(is_equal of the token's window-relative id against an iota row), and the PE
accumulates onehot^T @ [feats | 1] into a per-window [128, 66] PSUM bank in
f32 (bf16 multiplicands: features round once to bf16, counts are exact).  A
DVE epilogue divides sums by max(count, 1) and DMAs the window's 128 output
rows.  No GPSIMD scatter ucode anywhere — the old dma_scatter_add version was
descriptor-generation bound at ~7 ns/token.
"""
import os
import sys
from dataclasses import dataclass

sys.path.insert(0, "/opt/trn_rl_repo")

import numpy as np

NCORES = 8
C = 64
CW = 66  # 64 feats + count + window-relative id
W = 128  # ids per window


@dataclass(frozen=True)
class Cfg:
    n_coarse_pad: int = 250_880  # 8 * 245 * 128
    cap: int = 9                 # tiles of 128 tokens per window
    load_windows: int = 8        # windows per input DMA

    @property
    def rng(self):
        return self.n_coarse_pad // NCORES

    @property
    def n_win(self):  # windows per core
        return self.rng // W

    @property
    def s_slots(self):  # 128-token slots per core
        return self.n_win * self.cap

    @property
    def s_tot(self):
        return self.s_slots * 128


FUSED_OH = bool(int(os.environ.get("KERNEL_FUSED_OH", "1")))

CFG = Cfg()
_nc_cache = {}
LAST_RESULT = None


def build_nc(cfg: Cfg):
    from concourse import bacc, mybir, tile

    bf16 = mybir.dt.bfloat16
    f32 = mybir.dt.float32
    nc = bacc.Bacc("TRN2", target_bir_lowering=False)
    feats_ext = nc.declare_dram_parameter(
        "feats", [128, cfg.s_slots, CW], bf16, isOutput=False
    )
    iota_ext = nc.declare_dram_parameter("iota", [128, W], bf16, isOutput=False)
    out_ext = nc.declare_dram_parameter(
        "out", [cfg.n_win, W, C], f32, isOutput=True
    )

    lw = cfg.load_windows
    n_chunks = (cfg.n_win + lw - 1) // lw
    assert cfg.n_win % lw == 0 or True

    with tile.TileContext(nc) as tc:
        with (
            tc.tile_pool(name="stage", bufs=2) as stagep,
            tc.tile_pool(name="oh", bufs=4) as ohp,
            tc.tile_pool(name="psum", bufs=8, space="PSUM") as psump,
            tc.tile_pool(name="fin", bufs=4) as finp,
            tc.tile_pool(name="cst", bufs=1) as cstp,
        ):
            iota_t = cstp.tile([128, W], bf16)
            nc.sync.dma_start(out=iota_t[:], in_=iota_ext[:])

            for ch in range(n_chunks):
                w0 = ch * lw
                nw = min(lw, cfg.n_win - w0)
                src = stagep.tile([128, lw * cfg.cap, CW], bf16, tag="src")
                nc.sync.dma_start(
                    out=src[:, : nw * cfg.cap, :],
                    in_=feats_ext[:, w0 * cfg.cap : (w0 + nw) * cfg.cap, :],
                )
                for wi in range(nw):
                    w = w0 + wi
                    ps = psump.tile([128, CW], f32, tag="ps")
                    if FUSED_OH:
                        s0 = wi * cfg.cap
                        ohw = ohp.tile([128, cfg.cap, W], bf16, tag="ohw")
                        nc.vector.tensor_tensor(
                            out=ohw[:],
                            in0=src[:, s0 : s0 + cfg.cap, CW - 1 : CW].to_broadcast(
                                [128, cfg.cap, W]
                            ),
                            in1=iota_t[:].unsqueeze(1).to_broadcast(
                                [128, cfg.cap, W]
                            ),
                            op=mybir.AluOpType.is_equal,
                        )
                    for j in range(cfg.cap):
                        s = wi * cfg.cap + j
                        if FUSED_OH:
                            oh = ohw[:, j, :]
                        else:
                            oht = ohp.tile([128, W], bf16, tag="oh")
                            nc.vector.tensor_tensor(
                                out=oht[:],
                                in0=src[:, s, CW - 1 : CW].to_broadcast([128, W]),
                                in1=iota_t[:],
                                op=mybir.AluOpType.is_equal,
                            )
                            oh = oht[:]
                        nc.tensor.matmul(
                            out=ps[:],
                            lhsT=oh,
                            rhs=src[:, s, :CW],
                            start=(j == 0),
                            stop=(j == cfg.cap - 1),
                        )
                    den = finp.tile([128, 1], f32, tag="den")
                    nc.vector.tensor_scalar_max(den[:], ps[:, C : C + 1], 1.0)
                    inv = finp.tile([128, 1], f32, tag="inv")
                    nc.vector.reciprocal(inv[:], den[:])
                    ot = finp.tile([128, C], f32, tag="ot")
                    # multiply on the otherwise-idle ACT engine:
                    # out = Copy(in * scale), scale broadcast per partition
                    nc.scalar.activation(
                        ot[:], ps[:, :C], mybir.ActivationFunctionType.Copy,
                        scale=inv[:],
                    )
                    nc.sync.dma_start(out=out_ext[w], in_=ot[:])
    nc.compile()
    return nc


def shard_inputs(feats, ids, cfg: Cfg):
    """Host: route rows to owner cores, bucket into 128-id windows."""
    import ml_dtypes

    ids = np.asarray(ids, dtype=np.int64).ravel()
    feats = np.asarray(feats, dtype=np.float32)
    owner = ids // cfg.rng
    local = (ids - owner * cfg.rng).astype(np.int32)
    order = np.argsort(owner, kind="stable")
    counts = np.bincount(owner, minlength=NCORES)
    offs = np.zeros(NCORES + 1, np.int64)
    np.cumsum(counts, out=offs[1:])
    feats_sorted = feats[order]
    local_sorted = local[order]

    in_maps = []
    iota = np.broadcast_to(
        np.arange(W, dtype=np.float32), (128, W)
    ).astype(ml_dtypes.bfloat16)
    for k in range(NCORES):
        fk = feats_sorted[offs[k] : offs[k + 1]]
        lk = local_sorted[offs[k] : offs[k + 1]]
        n_k = lk.shape[0]
        fa = np.zeros((cfg.s_tot, CW), np.float32)
        if n_k:
            sorder = np.argsort(lk, kind="stable")
            ls = lk[sorder]
            win = ls >> 7
            wcount = np.bincount(win, minlength=cfg.n_win)
            assert wcount.max() <= cfg.cap * 128, (
                f"window overflow {wcount.max()} > {cfg.cap * 128}"
            )
            wstart = np.zeros(cfg.n_win, np.int64)
            np.cumsum(wcount[:-1], out=wstart[1:])
            rank_in_win = np.arange(n_k) - wstart[win]
            dst = win * (cfg.cap * 128) + rank_in_win
            fa[dst, :C] = fk[sorder]
            fa[dst, C] = 1.0
            fa[dst, C + 1] = (ls & 127).astype(np.float32)
        arranged = np.ascontiguousarray(
            fa.reshape(cfg.s_slots, 128, CW).transpose(1, 0, 2)
        ).astype(ml_dtypes.bfloat16)
        in_maps.append({"feats": arranged, "iota": iota})
    return in_maps


def assemble_output(results, n_coarse, cfg: Cfg):
    out = np.empty((NCORES * cfg.rng, C), np.float32)
    for k in range(NCORES):
        out[k * cfg.rng : (k + 1) * cfg.rng] = results[k]["out"].reshape(
            cfg.rng, C
        )
    return out[:n_coarse]


def emulate_device(in_map, cfg: Cfg):
    feats = np.asarray(in_map["feats"], dtype=np.float32)  # [128, s_slots, CW]
    acc = np.zeros((cfg.n_win, W, CW - 1), np.float64)
    for s in range(cfg.s_slots):
        w = s // cfg.cap
        for p in range(128):
            row = feats[p, s]
            seg = int(row[CW - 1])
            acc[w, seg, :] += row[: CW - 1]
    den = np.maximum(acc[:, :, C], 1.0)[:, :, None]
    return {"out": (acc[:, :, :C] / den).astype(np.float32)}


def _install_axon_hooks_shim():
    """Provide antenv.axon_hooks + the ctypes NTFF hook if the image lacks it.

    Mirrors trn_agent_boot.trn_boot._ntff_profile_via_ctypes so that
    run_bass_kernel_spmd(trace=True) can profile under axon.
    """
    import contextlib
    import ctypes
    import types

    try:
        from antenv.axon_hooks import get_axon_ntff_profile_hook  # noqa: F401

        return
    except ImportError:
        pass
    import antenv

    mod = types.ModuleType("antenv.axon_hooks")
    state = {"h": None}
    mod.set_axon_ntff_profile_hook = lambda h: state.__setitem__("h", h)
    mod.get_axon_ntff_profile_hook = lambda: state["h"]
    antenv.axon_hooks = mod
    sys.modules["antenv.axon_hooks"] = mod

    so_path = "/opt/axon/libaxon_pjrt.so"
    if not os.path.exists(so_path):
        return
    lib = ctypes.CDLL(so_path)
    if not hasattr(lib, "axon_start_nrt_profile"):
        return
    lib.axon_start_nrt_profile.argtypes = [
        ctypes.POINTER(ctypes.c_int64),
        ctypes.c_size_t,
    ]
    lib.axon_start_nrt_profile.restype = ctypes.c_int64
    lib.axon_stop_nrt_profile.argtypes = [ctypes.c_char_p]
    lib.axon_stop_nrt_profile.restype = ctypes.c_int64

    @contextlib.contextmanager
    def _hook(output_dir, device_ids):
        import jax

        jax.devices()
        if device_ids:
            ids = (ctypes.c_int64 * len(device_ids))(*device_ids)
            rc = lib.axon_start_nrt_profile(ids, len(device_ids))
        else:
            rc = lib.axon_start_nrt_profile(None, 0)
        if rc != 0:
            raise RuntimeError(f"axon_start_nrt_profile rc={rc}")
        try:
            yield
        finally:
            n = lib.axon_stop_nrt_profile(str(output_dir).encode())
            print(f"profile: {n} file(s) written to {output_dir}", file=sys.stderr)

    state["h"] = _hook


def kernel(fine_feats, coarse_ids, num_coarse):
    global LAST_RESULT
    from concourse.bass_utils import run_bass_kernel_spmd

    cfg = CFG
    # adapt window capacity to the data (stays at the default for the
    # expected uniform-random ids; protects other distributions)
    ids64 = np.asarray(coarse_ids, dtype=np.int64).ravel()
    owner = ids64 // cfg.rng
    local = ids64 - owner * cfg.rng
    mx = 0
    for k in range(NCORES):
        lk = local[owner == k]
        if lk.size:
            mx = max(mx, int(np.bincount(lk >> 7, minlength=cfg.n_win).max()))
    need_cap = max(cfg.cap, -(-mx // 128))
    if need_cap != cfg.cap:
        cfg = Cfg(cap=need_cap)
    in_maps = shard_inputs(fine_feats, coarse_ids, cfg)
    key = ("full", cfg.cap)
    if key not in _nc_cache:
        _nc_cache[key] = build_nc(cfg)
    nc = _nc_cache[key]
    trace = bool(int(os.environ.get("KERNEL_TRACE", "0")))
    if trace:
        _install_axon_hooks_shim()
    res = run_bass_kernel_spmd(nc, in_maps, core_ids=list(range(NCORES)), trace=trace)
    LAST_RESULT = res
    return assemble_output(res.results, int(num_coarse), cfg)



# revision 3
# speedup vs baseline: 1.5998x; 1.5998x over previous
"""Sparse avg-pool (segment mean) for Trainium2, 8 NeuronCores — pair-profile version.

Range-shard coarse ids across cores (core k owns ids [k*31360, (k+1)*31360)),
so no collective is needed.  Each core's 31360 ids form 245 windows of 128
consecutive ids.  The segment-sum runs on the TensorEngine as
onehot^T @ feats accumulated per window in PSUM.

The onehot build is the expensive part on the DVE, so the host arranges
tokens to amortize it: within a window, tokens of each id are padded to an
even count and dealt two-at-a-time to the same partition of a *pair* of
adjacent slots.  Both slots of a pair then share one id-profile prof[p], so a
single `tensor_scalar is_equal` (iota row vs per-partition f32 id scalar —
all unit-stride operands, 4x DVE mode) builds the onehot for two matmuls.

The host also pre-scales each token's features by 1/count(segment) (exact
int bincount), so PSUM accumulates the mean directly — no count column and
no divide epilogue.  The ACT engine copies PSUM to SBUF as bf16 and the
output DMAs out in multi-window chunks.
"""
import os
import sys
from dataclasses import dataclass

sys.path.insert(0, "/opt/trn_rl_repo")

import numpy as np

NCORES = 8
C = 64
W = 128          # ids per window
S = 2            # slots per group (pair): one onehot serves S matmuls
N_WIN = 245      # windows per core
RNG = N_WIN * W  # ids per core (31360)
N_COARSE_PAD = NCORES * RNG


@dataclass(frozen=True)
class Cfg:
    groups: int = 5      # pair-groups per window
    chunk_w: int = 7     # windows per DMA chunk (245 = 35 * 7)

    @property
    def slots_per_win(self):
        return self.groups * S

    @property
    def s_tot(self):  # slots per core
        return N_WIN * self.slots_per_win

    @property
    def n_chunks(self):
        return N_WIN // self.chunk_w


CFG = Cfg()
_nc_cache = {}
LAST_RESULT = None


def build_nc(cfg: Cfg):
    from concourse import bacc, mybir, tile

    bf16 = mybir.dt.bfloat16
    f32 = mybir.dt.float32
    nc = bacc.Bacc("TRN2", target_bir_lowering=False)
    feats_ext = nc.declare_dram_parameter(
        "feats", [128, cfg.s_tot, C], bf16, isOutput=False
    )
    ids_ext = nc.declare_dram_parameter(
        "ids", [128, N_WIN * cfg.groups], f32, isOutput=False
    )
    iota_ext = nc.declare_dram_parameter("iota", [128, W], bf16, isOutput=False)
    out_ext = nc.declare_dram_parameter(
        "out", [cfg.n_chunks, 128, cfg.chunk_w, C], bf16, isOutput=True
    )

    spw = cfg.slots_per_win
    with tile.TileContext(nc) as tc:
        with (
            tc.tile_pool(name="stage", bufs=2) as stagep,
            tc.tile_pool(name="oh", bufs=6) as ohp,
            tc.tile_pool(name="psum", bufs=8, space="PSUM") as psump,
            tc.tile_pool(name="fin", bufs=2) as finp,
            tc.tile_pool(name="cst", bufs=1) as cstp,
        ):
            iota_t = cstp.tile([128, W], bf16)
            nc.sync.dma_start(out=iota_t[:], in_=iota_ext[:])
            ids_t = cstp.tile([128, N_WIN * cfg.groups], f32)
            nc.sync.dma_start(out=ids_t[:], in_=ids_ext[:])

            for ch in range(cfg.n_chunks):
                src = stagep.tile([128, cfg.chunk_w * spw, C], bf16, tag="src")
                nc.sync.dma_start(
                    out=src[:],
                    in_=feats_ext[
                        :, ch * cfg.chunk_w * spw : (ch + 1) * cfg.chunk_w * spw, :
                    ],
                )
                ot = finp.tile([128, cfg.chunk_w, C], bf16, tag="ot")
                for wi in range(cfg.chunk_w):
                    w = ch * cfg.chunk_w + wi
                    ps = psump.tile([128, C], f32, tag="ps")
                    for g in range(cfg.groups):
                        oh = ohp.tile([128, W], bf16, tag="oh")
                        nc.vector.tensor_scalar(
                            out=oh[:],
                            in0=iota_t[:],
                            scalar1=ids_t[:, w * cfg.groups + g : w * cfg.groups + g + 1],
                            scalar2=None,
                            op0=mybir.AluOpType.is_equal,
                        )
                        for s in range(S):
                            slot = (wi * cfg.groups + g) * S + s
                            nc.tensor.matmul(
                                out=ps[:],
                                lhsT=oh[:],
                                rhs=src[:, slot, :],
                                start=(g == 0 and s == 0),
                                stop=(g == cfg.groups - 1 and s == S - 1),
                            )
                    nc.scalar.activation(
                        ot[:, wi, :], ps[:], mybir.ActivationFunctionType.Copy
                    )
                nc.sync.dma_start(out=out_ext[ch], in_=ot[:])
    nc.compile()
    return nc


def shard_inputs(feats, ids, cfg: Cfg):
    """Host: route rows to owner cores, pair-pack into shared-profile groups."""
    import ml_dtypes

    ids = np.asarray(ids, dtype=np.int64).ravel()
    feats = np.asarray(feats, dtype=np.float32)
    n = ids.shape[0]

    # pre-scale by 1/count so the device sums means directly
    cnt = np.bincount(ids, minlength=N_COARSE_PAD).astype(np.float32)
    scale = 1.0 / np.maximum(cnt, 1.0)
    feats_scaled = feats * scale[ids][:, None]

    owner = ids // RNG
    local = (ids - owner * RNG).astype(np.int64)
    order = np.argsort(owner, kind="stable")
    counts = np.bincount(owner, minlength=NCORES)
    offs = np.zeros(NCORES + 1, np.int64)
    np.cumsum(counts, out=offs[1:])
    feats_sorted = feats_scaled[order]
    local_sorted = local[order]

    iota = np.broadcast_to(
        np.arange(W, dtype=np.float32), (128, W)
    ).astype(ml_dtypes.bfloat16)

    in_maps = []
    for k in range(NCORES):
        fk = feats_sorted[offs[k] : offs[k + 1]]
        lk = local_sorted[offs[k] : offs[k + 1]]
        n_k = lk.shape[0]
        fa = np.zeros((128, cfg.s_tot, C), np.float32)
        prof = np.zeros((128, N_WIN * cfg.groups), np.float32)
        if n_k:
            sorder = np.argsort(lk, kind="stable")
            ls = lk[sorder]
            fs = fk[sorder]
            win = ls >> 7
            wrel = (ls & 127).astype(np.float32)
            # rank of each token within its id
            first_idx = np.searchsorted(ls, ls, side="left")
            rank = np.arange(n_k) - first_idx
            cell_of_tok = rank // S          # cell index within the id
            j = rank % S                     # slot position within the pair
            # enumerate cells in sorted order; find each cell's rank in its window
            is_cell_start = j == 0
            cell_global = np.cumsum(is_cell_start) - 1
            win_of_cell = win[is_cell_start]
            cell_start_per_win = np.searchsorted(win_of_cell, np.arange(N_WIN))
            cell_rank = cell_global - cell_start_per_win[win]
            p = cell_rank % 128
            g = cell_rank // 128
            assert g.max() < cfg.groups, f"group overflow {g.max()+1} > {cfg.groups}"
            slot = win * (cfg.groups * S) + g * S + j
            fa[p, slot, :] = fs
            prof[p, win * cfg.groups + g] = wrel
        in_maps.append(
            {
                "feats": fa.astype(ml_dtypes.bfloat16),
                "ids": prof,
                "iota": iota,
            }
        )
    return in_maps


def assemble_output(results, n_coarse, cfg: Cfg):
    out = np.empty((NCORES * RNG, C), np.float32)
    for k in range(NCORES):
        r = np.asarray(results[k]["out"], dtype=np.float32)  # [n_chunks,128,chunk_w,C]
        out[k * RNG : (k + 1) * RNG] = r.transpose(0, 2, 1, 3).reshape(RNG, C)
    return out[:n_coarse]


def _needed_groups(coarse_ids):
    """Max pair-cells in any (core, window), in units of 128-partition groups."""
    ids = np.asarray(coarse_ids, dtype=np.int64).ravel()
    # cells per (window, id) = ceil(count/S); sum per window
    cnt_per_id = np.bincount(ids, minlength=N_COARSE_PAD)
    cells_per_id = -(-cnt_per_id // S)
    cells_per_win = cells_per_id.reshape(NCORES * N_WIN, W).sum(axis=1)
    mx = int(cells_per_win.max()) if cells_per_win.size else 0
    return max(1, -(-mx // 128))


def _install_axon_hooks_shim():
    """Provide antenv.axon_hooks + the ctypes NTFF hook if the image lacks it."""
    import contextlib
    import ctypes
    import types

    try:
        from antenv.axon_hooks import get_axon_ntff_profile_hook  # noqa: F401

        return
    except ImportError:
        pass
    import antenv

    mod = types.ModuleType("antenv.axon_hooks")
    state = {"h": None}
    mod.set_axon_ntff_profile_hook = lambda h: state.__setitem__("h", h)
    mod.get_axon_ntff_profile_hook = lambda: state["h"]
    antenv.axon_hooks = mod
    sys.modules["antenv.axon_hooks"] = mod

    so_path = "/opt/axon/libaxon_pjrt.so"
    if not os.path.exists(so_path):
        return
    lib = ctypes.CDLL(so_path)
    if not hasattr(lib, "axon_start_nrt_profile"):
        return
    lib.axon_start_nrt_profile.argtypes = [
        ctypes.POINTER(ctypes.c_int64),
        ctypes.c_size_t,
    ]
    lib.axon_start_nrt_profile.restype = ctypes.c_int64
    lib.axon_stop_nrt_profile.argtypes = [ctypes.c_char_p]
    lib.axon_stop_nrt_profile.restype = ctypes.c_int64

    @contextlib.contextmanager
    def _hook(output_dir, device_ids):
        import jax

        jax.devices()
        if device_ids:
            idsv = (ctypes.c_int64 * len(device_ids))(*device_ids)
            rc = lib.axon_start_nrt_profile(idsv, len(device_ids))
        else:
            rc = lib.axon_start_nrt_profile(None, 0)
        if rc != 0:
            raise RuntimeError(f"axon_start_nrt_profile rc={rc}")
        try:
            yield
        finally:
            nfiles = lib.axon_stop_nrt_profile(str(output_dir).encode())
            print(f"profile: {nfiles} file(s) written to {output_dir}", file=sys.stderr)

    state["h"] = _hook


def kernel(fine_feats, coarse_ids, num_coarse):
    global LAST_RESULT
    from concourse.bass_utils import run_bass_kernel_spmd

    cfg = CFG
    need_g = _needed_groups(coarse_ids)
    if need_g > cfg.groups:
        cfg = Cfg(groups=need_g)
    in_maps = shard_inputs(fine_feats, coarse_ids, cfg)
    key = ("pair", cfg.groups)
    if key not in _nc_cache:
        _nc_cache[key] = build_nc(cfg)
    nc = _nc_cache[key]
    trace = bool(int(os.environ.get("KERNEL_TRACE", "0")))
    if trace:
        _install_axon_hooks_shim()
    res = run_bass_kernel_spmd(nc, in_maps, core_ids=list(range(NCORES)), trace=trace)
    LAST_RESULT = res
    return assemble_output(res.results, int(num_coarse), cfg)


# revision 11
# speedup vs baseline: 1.7781x; 1.1114x over previous
"""Sparse avg-pool (segment mean) for Trainium2, 8 NeuronCores — grouped-profile version.

Range-shard coarse ids across cores (core k owns ids [k*31360, (k+1)*31360)),
so no collective is needed.  Each core's 31360 ids form 245 windows of 128
consecutive ids.  The segment-sum runs on the TensorEngine as
onehot^T @ feats accumulated per window in PSUM.

The onehot build is the DVE bottleneck (~163ns per tensor_scalar with an AP
scalar, regardless of size), so the packing amortizes one onehot over as
many matmul slots as possible: a *group* is a set of S slots sharing one
id-profile prof[p] — partition p of every slot in the group holds tokens of
window-relative id prof[p].  The host greedily decomposes each window's id
counts into one S=5 group plus one S=4 group (capacity 9*128=1152 vs ~1020
tokens), with rare overflow groups.  One `tensor_scalar is_equal` per group
(iota row vs per-partition f32 id — unit-stride bf16 operands, 4x DVE mode)
builds the onehot for all S matmuls of the group.

The group structure is data-dependent and baked into the program at build
time; the structure is unioned across the 8 cores so a single SPMD program
serves all of them.  Features are pre-scaled by 1/count on the host (exact
int bincount), so PSUM accumulates the mean directly, and the output is
written back as bf16 to halve the output DMA.
"""
import os
import sys

sys.path.insert(0, "/opt/trn_rl_repo")

import numpy as np

NCORES = 8
C = 64
W = 128          # ids per window
N_WIN = 245      # windows per core
RNG = N_WIN * W  # ids per core (31360)
N_COARSE_PAD = NCORES * RNG
CHUNK_W = 7      # windows per DMA chunk (245 = 35 * 7)

_nc_cache = {}
LAST_RESULT = None


# --------------------------------------------------------------------------
# host-side packing
# --------------------------------------------------------------------------

def _reduce_to_budget(x, l, A, B):
    """Convert A-cells to leftover until sum(x) <= 128, preferring ids where
    the extra A tokens add the fewest B-cells.  Mutates x, l; returns ok."""
    over = int(x.sum()) - 128
    while over > 0:
        cand = np.nonzero(x > 0)[0]
        if cand.size == 0:
            return False
        delta = (-(-(l[cand] + A) // B)) - (-(-l[cand] // B))
        i = int(cand[np.argmin(delta)])
        x[i] -= 1
        l[i] += A
        over -= 1
    return True


def _emit_cells(per_id, S):
    """per_id[i] = token count to place in <=S-token cells of id i."""
    out = []
    for i in np.nonzero(per_id)[0]:
        ci = int(per_id[i])
        while ci > 0:
            m = min(S, ci)
            out.append((int(i), m))
            ci -= m
    return out


def _fit_sizes(c, sizes):
    """Decompose counts c into groups of the given sizes (desc).  Returns
    cells-per-group or None."""
    l = c.astype(np.int64).copy()
    groups = []
    for j, S in enumerate(sizes):
        nxt = sizes[j + 1] if j + 1 < len(sizes) else None
        x = l // S
        rem = l - S * x
        if nxt is None:
            if int((-(-l // S)).sum()) > 128:
                return None
            groups.append(_emit_cells(l, S))
            l = np.zeros_like(l)
        else:
            if not _reduce_to_budget(x, rem, S, nxt):
                return None
            groups.append(_emit_cells(x * S, S))
            l = rem
    if int(l.sum()) != 0:
        return None
    return groups


MENU = [(5, 3), (5, 4), (6, 4), (6, 5), (6, 5, 2), (6, 5, 5)]


def _cells_5x(c):
    """Fallback: (5,5,...) decomposition, biggest cells first."""
    cells = _emit_cells(c, 5)
    cells.sort(key=lambda t: -t[1])
    groups = [cells[j : j + 128] for j in range(0, len(cells), 128)]
    if not groups:
        groups = [[]]
    sizes = [max((m for _, m in g), default=1) for g in groups]
    return sizes, groups, len(MENU)


def _decomp_window(counts):
    """Returns (sizes, cells_per_group, menu_rank)."""
    for rank, sizes in enumerate(MENU):
        g = _fit_sizes(counts, list(sizes))
        if g is not None:
            return list(sizes), g, rank
    return _cells_5x(counts)


def build_structure(per_core_counts):
    """Per-core decompositions, heaviness-sorted window alignment, union sizes.

    Returns (structure, cells, perm):
      structure[w] = union group sizes at aligned position w
      cells[k][w]  = cell lists for core k's window at position w
      perm[k][w]   = the actual window index of core k at position w
    """
    cells = [[None] * N_WIN for _ in range(NCORES)]
    perm = np.zeros((NCORES, N_WIN), np.int64)
    for k in range(NCORES):
        rows = []
        for w in range(N_WIN):
            c = per_core_counts[k][w]
            sizes_k, cells_k, rank = _decomp_window(c)
            rows.append((rank, -int(c.sum()), w, sizes_k, cells_k))
        rows.sort()
        for pos, (_, _, w, sizes_k, cells_k) in enumerate(rows):
            perm[k][pos] = w
            cells[k][pos] = (sizes_k, cells_k)
    structure = []
    for pos in range(N_WIN):
        union = []
        for k in range(NCORES):
            for j, s in enumerate(cells[k][pos][0]):
                if j < len(union):
                    union[j] = max(union[j], s)
                else:
                    union.append(s)
        if not union:
            union = [1]
        structure.append(union)
    return structure, cells, perm


class Layout:
    """Slot/group offsets derived from the union structure."""

    def __init__(self, structure):
        self.structure = structure
        self.win_slot_off = np.zeros(N_WIN + 1, np.int64)
        self.win_grp_off = np.zeros(N_WIN + 1, np.int64)
        for w, sizes in enumerate(structure):
            self.win_slot_off[w + 1] = self.win_slot_off[w] + sum(sizes)
            self.win_grp_off[w + 1] = self.win_grp_off[w] + len(sizes)
        self.s_tot = int(self.win_slot_off[-1])
        self.g_tot = int(self.win_grp_off[-1])


def shard_inputs(feats, ids, layout, cells, perm):
    """Route rows to owner cores and place tokens per the precomputed cells.

    Builds, per core, index arrays (partition, slot) for every token and does
    one vectorized scatter of the pre-scaled features.
    """
    import ml_dtypes

    ids = np.asarray(ids, dtype=np.int64).ravel()
    feats = np.asarray(feats, dtype=np.float32)

    cnt = np.bincount(ids, minlength=N_COARSE_PAD).astype(np.float32)
    scale = 1.0 / np.maximum(cnt, 1.0)
    feats_scaled = feats * scale[ids][:, None]

    owner = ids // RNG
    order = np.argsort(owner, kind="stable")
    counts_per_core = np.bincount(owner, minlength=NCORES)
    offs = np.zeros(NCORES + 1, np.int64)
    np.cumsum(counts_per_core, out=offs[1:])
    feats_sorted = feats_scaled[order]
    local_sorted = (ids - owner * RNG)[order]

    iota = np.broadcast_to(
        np.arange(W, dtype=np.float32), (128, W)
    ).astype(ml_dtypes.bfloat16)

    in_maps = []
    for k in range(NCORES):
        fk = feats_sorted[offs[k] : offs[k + 1]]
        lk = local_sorted[offs[k] : offs[k + 1]]
        fa = np.zeros((128, layout.s_tot, C), np.float32)
        prof = np.zeros((128, layout.g_tot), np.float32)
        if lk.shape[0]:
            sorder = np.argsort(lk, kind="stable")
            ls = lk[sorder]
            fs = fk[sorder]
            win = ls >> 7
            wstart = np.searchsorted(win, np.arange(N_WIN + 1))
            dst_p = np.empty(ls.shape[0], np.int64)
            dst_s = np.empty(ls.shape[0], np.int64)
            for pos in range(N_WIN):
                w = int(perm[k][pos])
                lo, hi = int(wstart[w]), int(wstart[w + 1])
                if lo == hi:
                    continue
                wrel = (ls[lo:hi] & 127).astype(np.int64)
                worder = np.argsort(wrel, kind="stable")
                counts = np.bincount(wrel, minlength=W)
                id_start = np.zeros(W + 1, np.int64)
                np.cumsum(counts, out=id_start[1:])
                used = np.zeros(W, np.int64)
                slot0 = int(layout.win_slot_off[pos])
                grp0 = int(layout.win_grp_off[pos])
                s_off = 0
                gcells_list = cells[k][pos][1]
                for g, S in enumerate(layout.structure[pos]):
                    gcells = gcells_list[g] if g < len(gcells_list) else []
                    for p, (i, m) in enumerate(gcells):
                        prof[p, grp0 + g] = i
                        u = used[i]
                        toks = worder[id_start[i] + u : id_start[i] + u + m]
                        used[i] = u + m
                        dst_p[lo + toks] = p
                        dst_s[lo + toks] = slot0 + s_off + np.arange(m)
                    s_off += S
            fa[dst_p, dst_s, :] = fs
        in_maps.append(
            {
                "feats": fa.astype(ml_dtypes.bfloat16),
                "ids": prof,
                "iota": iota,
            }
        )
    return in_maps


# --------------------------------------------------------------------------
# device program
# --------------------------------------------------------------------------

def build_nc(layout):
    from concourse import bacc, mybir, tile

    bf16 = mybir.dt.bfloat16
    f32 = mybir.dt.float32
    nc = bacc.Bacc("TRN2", target_bir_lowering=False)
    feats_ext = nc.declare_dram_parameter(
        "feats", [128, layout.s_tot, C], bf16, isOutput=False
    )
    ids_ext = nc.declare_dram_parameter("ids", [128, layout.g_tot], f32, isOutput=False)
    iota_ext = nc.declare_dram_parameter("iota", [128, W], bf16, isOutput=False)
    n_chunks = N_WIN // CHUNK_W
    out_ext = nc.declare_dram_parameter(
        "out", [n_chunks, 128, CHUNK_W, C], bf16, isOutput=True
    )

    # chunk slot extents
    chunk_lo = [int(layout.win_slot_off[ch * CHUNK_W]) for ch in range(n_chunks)]
    chunk_hi = [int(layout.win_slot_off[(ch + 1) * CHUNK_W]) for ch in range(n_chunks)]
    max_chunk_slots = max(hi - lo for lo, hi in zip(chunk_lo, chunk_hi))

    with tile.TileContext(nc) as tc:
        with (
            tc.tile_pool(name="stage", bufs=2) as stagep,
            tc.tile_pool(name="oh", bufs=6) as ohp,
            tc.tile_pool(name="psum", bufs=8, space="PSUM") as psump,
            tc.tile_pool(name="fin", bufs=2) as finp,
            tc.tile_pool(name="cst", bufs=1) as cstp,
        ):
            iota_t = cstp.tile([128, W], bf16)
            nc.sync.dma_start(out=iota_t[:], in_=iota_ext[:])
            ids_t = cstp.tile([128, layout.g_tot], f32)
            nc.sync.dma_start(out=ids_t[:], in_=ids_ext[:])

            for ch in range(n_chunks):
                lo, hi = chunk_lo[ch], chunk_hi[ch]
                src = stagep.tile([128, max_chunk_slots, C], bf16, tag="src")
                nc.sync.dma_start(
                    out=src[:, : hi - lo, :], in_=feats_ext[:, lo:hi, :]
                )
                ot = finp.tile([128, CHUNK_W, C], bf16, tag="ot")
                for wi in range(CHUNK_W):
                    w = ch * CHUNK_W + wi
                    sizes = layout.structure[w]
                    ps = psump.tile([128, C], f32, tag="ps")
                    n_slots = sum(sizes)
                    s_base = int(layout.win_slot_off[w]) - lo
                    g_base = int(layout.win_grp_off[w])
                    s_off = 0
                    for g, S in enumerate(sizes):
                        oh = ohp.tile([128, W], bf16, tag="oh")
                        nc.vector.tensor_scalar(
                            out=oh[:],
                            in0=iota_t[:],
                            scalar1=ids_t[:, g_base + g : g_base + g + 1],
                            scalar2=None,
                            op0=mybir.AluOpType.is_equal,
                        )
                        for s in range(S):
                            slot = s_base + s_off + s
                            nc.tensor.matmul(
                                out=ps[:],
                                lhsT=oh[:],
                                rhs=src[:, slot, :],
                                start=(s_off + s == 0),
                                stop=(s_off + s == n_slots - 1),
                            )
                        s_off += S
                    nc.scalar.activation(
                        ot[:, wi, :], ps[:], mybir.ActivationFunctionType.Copy
                    )
                nc.sync.dma_start(out=out_ext[ch], in_=ot[:])
    nc.compile()
    return nc


def assemble_output(results, n_coarse, perm):
    out = np.empty((NCORES * RNG, C), np.float32)
    for k in range(NCORES):
        r = np.asarray(results[k]["out"], dtype=np.float32)  # [n_chunks,128,CHUNK_W,C]
        by_pos = r.transpose(0, 2, 1, 3).reshape(N_WIN, W, C)
        core = out[k * RNG : (k + 1) * RNG].reshape(N_WIN, W, C)
        core[perm[k]] = by_pos
    return out[:n_coarse]


def _install_axon_hooks_shim():
    """Provide antenv.axon_hooks + the ctypes NTFF hook if the image lacks it."""
    import contextlib
    import ctypes
    import types

    try:
        from antenv.axon_hooks import get_axon_ntff_profile_hook  # noqa: F401

        return
    except ImportError:
        pass
    import antenv

    mod = types.ModuleType("antenv.axon_hooks")
    state = {"h": None}
    mod.set_axon_ntff_profile_hook = lambda h: state.__setitem__("h", h)
    mod.get_axon_ntff_profile_hook = lambda: state["h"]
    antenv.axon_hooks = mod
    sys.modules["antenv.axon_hooks"] = mod

    so_path = "/opt/axon/libaxon_pjrt.so"
    if not os.path.exists(so_path):
        return
    lib = ctypes.CDLL(so_path)
    if not hasattr(lib, "axon_start_nrt_profile"):
        return
    lib.axon_start_nrt_profile.argtypes = [
        ctypes.POINTER(ctypes.c_int64),
        ctypes.c_size_t,
    ]
    lib.axon_start_nrt_profile.restype = ctypes.c_int64
    lib.axon_stop_nrt_profile.argtypes = [ctypes.c_char_p]
    lib.axon_stop_nrt_profile.restype = ctypes.c_int64

    @contextlib.contextmanager
    def _hook(output_dir, device_ids):
        import jax

        jax.devices()
        if device_ids:
            idsv = (ctypes.c_int64 * len(device_ids))(*device_ids)
            rc = lib.axon_start_nrt_profile(idsv, len(device_ids))
        else:
            rc = lib.axon_start_nrt_profile(None, 0)
        if rc != 0:
            raise RuntimeError(f"axon_start_nrt_profile rc={rc}")
        try:
            yield
        finally:
            nfiles = lib.axon_stop_nrt_profile(str(output_dir).encode())
            print(f"profile: {nfiles} file(s) written to {output_dir}", file=sys.stderr)

    state["h"] = _hook


def _per_core_counts(coarse_ids):
    """[NCORES][N_WIN][W] token counts."""
    ids = np.asarray(coarse_ids, dtype=np.int64).ravel()
    cnt = np.bincount(ids, minlength=N_COARSE_PAD)
    return cnt.reshape(NCORES, N_WIN, W)


def kernel(fine_feats, coarse_ids, num_coarse):
    global LAST_RESULT
    from concourse.bass_utils import run_bass_kernel_spmd

    counts = _per_core_counts(coarse_ids)
    structure, cells, perm = build_structure(counts)
    layout = Layout(structure)
    in_maps = shard_inputs(fine_feats, coarse_ids, layout, cells, perm)
    key = tuple(tuple(s) for s in structure)
    if key not in _nc_cache:
        _nc_cache.clear()
        _nc_cache[key] = build_nc(layout)
    nc = _nc_cache[key]
    trace = bool(int(os.environ.get("KERNEL_TRACE", "0")))
    if trace:
        _install_axon_hooks_shim()
    res = run_bass_kernel_spmd(nc, in_maps, core_ids=list(range(NCORES)), trace=trace)
    LAST_RESULT = res
    return assemble_output(res.results, int(num_coarse), perm)


# revision 13
# speedup vs baseline: 2.3820x; 1.3396x over previous
"""Sparse avg-pool (segment mean) for Trainium2, 8 NeuronCores — grouped-profile version.

Range-shard coarse ids across cores (core k owns ids [k*31360, (k+1)*31360)),
so no collective is needed.  Each core's 31360 ids form 245 windows of 128
consecutive ids.  The segment-sum runs on the TensorEngine as
onehot^T @ feats accumulated per window in PSUM.

The onehot build is the DVE bottleneck (~163ns per tensor_scalar with an AP
scalar, regardless of size), so the packing amortizes one onehot over as
many matmul slots as possible: a *group* is a set of S slots sharing one
id-profile prof[p] — partition p of every slot in the group holds tokens of
window-relative id prof[p].  The host greedily decomposes each window's id
counts into one S=5 group plus one S=4 group (capacity 9*128=1152 vs ~1020
tokens), with rare overflow groups.  One `tensor_scalar is_equal` per group
(iota row vs per-partition f32 id — unit-stride bf16 operands, 4x DVE mode)
builds the onehot for all S matmuls of the group.

The group structure is data-dependent and baked into the program at build
time; the structure is unioned across the 8 cores so a single SPMD program
serves all of them.  Features are pre-scaled by 1/count on the host (exact
int bincount), so PSUM accumulates the mean directly, and the output is
written back as bf16 to halve the output DMA.
"""
import os
import sys

sys.path.insert(0, "/opt/trn_rl_repo")

import numpy as np

NCORES = 8
C = 64
W = 128          # ids per window
N_WIN = 245      # windows per core
RNG = N_WIN * W  # ids per core (31360)
N_COARSE_PAD = NCORES * RNG
CHUNK_W = 7      # windows per DMA chunk (245 = 35 * 7)

_nc_cache = {}
LAST_RESULT = None


# --------------------------------------------------------------------------
# host-side packing
# --------------------------------------------------------------------------

def _reduce_to_budget(x, l, A, B):
    """Convert A-cells to leftover until sum(x) <= 128, preferring ids where
    the extra A tokens add the fewest B-cells.  Mutates x, l; returns ok."""
    over = int(x.sum()) - 128
    while over > 0:
        cand = np.nonzero(x > 0)[0]
        if cand.size == 0:
            return False
        delta = (-(-(l[cand] + A) // B)) - (-(-l[cand] // B))
        i = int(cand[np.argmin(delta)])
        x[i] -= 1
        l[i] += A
        over -= 1
    return True


def _emit_cells(per_id, S):
    """per_id[i] = token count to place in <=S-token cells of id i."""
    out = []
    for i in np.nonzero(per_id)[0]:
        ci = int(per_id[i])
        while ci > 0:
            m = min(S, ci)
            out.append((int(i), m))
            ci -= m
    return out


def _fit_sizes(c, sizes):
    """Decompose counts c into groups of the given sizes (desc).  Returns
    cells-per-group or None."""
    l = c.astype(np.int64).copy()
    groups = []
    for j, S in enumerate(sizes):
        nxt = sizes[j + 1] if j + 1 < len(sizes) else None
        x = l // S
        rem = l - S * x
        if nxt is None:
            if int((-(-l // S)).sum()) > 128:
                return None
            groups.append(_emit_cells(l, S))
            l = np.zeros_like(l)
        else:
            if not _reduce_to_budget(x, rem, S, nxt):
                return None
            groups.append(_emit_cells(x * S, S))
            l = rem
    if int(l.sum()) != 0:
        return None
    return groups


MENU = [(5, 3), (5, 4), (6, 4), (6, 5), (6, 5, 2), (6, 5, 5)]


def _cells_5x(c):
    """Fallback: (5,5,...) decomposition, biggest cells first."""
    cells = _emit_cells(c, 5)
    cells.sort(key=lambda t: -t[1])
    groups = [cells[j : j + 128] for j in range(0, len(cells), 128)]
    if not groups:
        groups = [[]]
    sizes = [max((m for _, m in g), default=1) for g in groups]
    return sizes, groups, len(MENU)


def _decomp_window(counts):
    """Returns (sizes, cells_per_group, menu_rank)."""
    for rank, sizes in enumerate(MENU):
        g = _fit_sizes(counts, list(sizes))
        if g is not None:
            return list(sizes), g, rank
    return _cells_5x(counts)


def build_structure(per_core_counts):
    """Per-core decompositions, heaviness-sorted window alignment, union sizes.

    Returns (structure, cells, perm):
      structure[w] = union group sizes at aligned position w
      cells[k][w]  = cell lists for core k's window at position w
      perm[k][w]   = the actual window index of core k at position w
    """
    cells = [[None] * N_WIN for _ in range(NCORES)]
    perm = np.zeros((NCORES, N_WIN), np.int64)
    for k in range(NCORES):
        rows = []
        for w in range(N_WIN):
            c = per_core_counts[k][w]
            sizes_k, cells_k, rank = _decomp_window(c)
            rows.append((rank, -int(c.sum()), w, sizes_k, cells_k))
        rows.sort()
        for pos, (_, _, w, sizes_k, cells_k) in enumerate(rows):
            perm[k][pos] = w
            cells[k][pos] = (sizes_k, cells_k)
    structure = []
    for pos in range(N_WIN):
        union = []
        for k in range(NCORES):
            for j, s in enumerate(cells[k][pos][0]):
                if j < len(union):
                    union[j] = max(union[j], s)
                else:
                    union.append(s)
        if not union:
            union = [1]
        structure.append(union)
    return structure, cells, perm


class Layout:
    """Slot/group offsets derived from the union structure."""

    def __init__(self, structure):
        self.structure = structure
        self.win_slot_off = np.zeros(N_WIN + 1, np.int64)
        self.win_grp_off = np.zeros(N_WIN + 1, np.int64)
        for w, sizes in enumerate(structure):
            self.win_slot_off[w + 1] = self.win_slot_off[w] + sum(sizes)
            self.win_grp_off[w + 1] = self.win_grp_off[w] + len(sizes)
        self.s_tot = int(self.win_slot_off[-1])
        self.g_tot = int(self.win_grp_off[-1])


def shard_inputs(feats, ids, layout, cells, perm):
    """Route rows to owner cores and place tokens per the precomputed cells.

    Builds, per core, index arrays (partition, slot) for every token and does
    one vectorized scatter of the pre-scaled features.
    """
    import ml_dtypes

    ids = np.asarray(ids, dtype=np.int64).ravel()
    feats = np.asarray(feats, dtype=np.float32)

    cnt = np.bincount(ids, minlength=N_COARSE_PAD).astype(np.float32)
    scale = 1.0 / np.maximum(cnt, 1.0)
    feats_scaled = feats * scale[ids][:, None]

    owner = ids // RNG
    order = np.argsort(owner, kind="stable")
    counts_per_core = np.bincount(owner, minlength=NCORES)
    offs = np.zeros(NCORES + 1, np.int64)
    np.cumsum(counts_per_core, out=offs[1:])
    feats_sorted = feats_scaled[order]
    local_sorted = (ids - owner * RNG)[order]

    iota = np.broadcast_to(
        np.arange(W, dtype=np.float32), (128, W)
    ).astype(ml_dtypes.bfloat16)

    in_maps = []
    for k in range(NCORES):
        fk = feats_sorted[offs[k] : offs[k + 1]]
        lk = local_sorted[offs[k] : offs[k + 1]]
        fa = np.zeros((128, layout.s_tot, C), np.float32)
        prof = np.zeros((128, layout.g_tot), np.float32)
        if lk.shape[0]:
            sorder = np.argsort(lk, kind="stable")
            ls = lk[sorder]
            fs = fk[sorder]
            win = ls >> 7
            wstart = np.searchsorted(win, np.arange(N_WIN + 1))
            dst_p = np.empty(ls.shape[0], np.int64)
            dst_s = np.empty(ls.shape[0], np.int64)
            for pos in range(N_WIN):
                w = int(perm[k][pos])
                lo, hi = int(wstart[w]), int(wstart[w + 1])
                if lo == hi:
                    continue
                wrel = (ls[lo:hi] & 127).astype(np.int64)
                worder = np.argsort(wrel, kind="stable")
                counts = np.bincount(wrel, minlength=W)
                id_start = np.zeros(W + 1, np.int64)
                np.cumsum(counts, out=id_start[1:])
                used = np.zeros(W, np.int64)
                slot0 = int(layout.win_slot_off[pos])
                grp0 = int(layout.win_grp_off[pos])
                s_off = 0
                gcells_list = cells[k][pos][1]
                for g, S in enumerate(layout.structure[pos]):
                    gcells = gcells_list[g] if g < len(gcells_list) else []
                    for p, (i, m) in enumerate(gcells):
                        prof[p, grp0 + g] = i
                        u = used[i]
                        toks = worder[id_start[i] + u : id_start[i] + u + m]
                        used[i] = u + m
                        dst_p[lo + toks] = p
                        dst_s[lo + toks] = slot0 + s_off + np.arange(m)
                    s_off += S
            fa[dst_p, dst_s, :] = fs
        in_maps.append(
            {
                "feats": fa.astype(ml_dtypes.bfloat16),
                "ids": prof,
                "iota": iota,
            }
        )
    return in_maps


# --------------------------------------------------------------------------
# device program
# --------------------------------------------------------------------------

def build_nc(layout):
    from concourse import bacc, mybir, tile

    bf16 = mybir.dt.bfloat16
    f32 = mybir.dt.float32
    nc = bacc.Bacc("TRN2", target_bir_lowering=False)
    feats_ext = nc.declare_dram_parameter(
        "feats", [128, layout.s_tot, C], bf16, isOutput=False
    )
    ids_ext = nc.declare_dram_parameter("ids", [128, layout.g_tot], f32, isOutput=False)
    iota_ext = nc.declare_dram_parameter("iota", [128, W], bf16, isOutput=False)
    n_chunks = N_WIN // CHUNK_W
    out_ext = nc.declare_dram_parameter(
        "out", [n_chunks, 128, CHUNK_W, C], bf16, isOutput=True
    )

    # chunk slot extents
    chunk_lo = [int(layout.win_slot_off[ch * CHUNK_W]) for ch in range(n_chunks)]
    chunk_hi = [int(layout.win_slot_off[(ch + 1) * CHUNK_W]) for ch in range(n_chunks)]
    max_chunk_slots = max(hi - lo for lo, hi in zip(chunk_lo, chunk_hi))

    with tile.TileContext(nc) as tc:
        with (
            tc.tile_pool(name="stage", bufs=3) as stagep,
            tc.tile_pool(name="oh", bufs=10) as ohp,
            tc.tile_pool(name="psum", bufs=8, space="PSUM") as psump,
            tc.tile_pool(name="fin", bufs=3) as finp,
            tc.tile_pool(name="cst", bufs=1) as cstp,
        ):
            iota_t = cstp.tile([128, W], bf16)
            nc.sync.dma_start(out=iota_t[:], in_=iota_ext[:])
            ids_t = cstp.tile([128, layout.g_tot], f32)
            nc.sync.dma_start(out=ids_t[:], in_=ids_ext[:])

            for ch in range(n_chunks):
                lo, hi = chunk_lo[ch], chunk_hi[ch]
                src = stagep.tile([128, max_chunk_slots, C], bf16, tag="src")
                nc.sync.dma_start(
                    out=src[:, : hi - lo, :], in_=feats_ext[:, lo:hi, :]
                )
                ot = finp.tile([128, CHUNK_W, C], bf16, tag="ot")
                for wi in range(CHUNK_W):
                    w = ch * CHUNK_W + wi
                    sizes = layout.structure[w]
                    ps = psump.tile([128, C], f32, tag="ps")
                    n_slots = sum(sizes)
                    s_base = int(layout.win_slot_off[w]) - lo
                    g_base = int(layout.win_grp_off[w])
                    s_off = 0
                    for g, S in enumerate(sizes):
                        oh = ohp.tile([128, W], bf16, tag="oh")
                        nc.vector.tensor_scalar(
                            out=oh[:],
                            in0=iota_t[:],
                            scalar1=ids_t[:, g_base + g : g_base + g + 1],
                            scalar2=None,
                            op0=mybir.AluOpType.is_equal,
                        )
                        for s in range(S):
                            slot = s_base + s_off + s
                            nc.tensor.matmul(
                                out=ps[:],
                                lhsT=oh[:],
                                rhs=src[:, slot, :],
                                start=(s_off + s == 0),
                                stop=(s_off + s == n_slots - 1),
                            )
                        s_off += S
                    nc.scalar.activation(
                        ot[:, wi, :], ps[:], mybir.ActivationFunctionType.Copy
                    )
                # output DMA on the idle Pool queue so it can't head-of-line
                # block the next chunk's input DMA on the Sync queue
                nc.gpsimd.dma_start(out=out_ext[ch], in_=ot[:])
    nc.compile()
    return nc


def assemble_output(results, n_coarse, perm):
    out = np.empty((NCORES * RNG, C), np.float32)
    for k in range(NCORES):
        r = np.asarray(results[k]["out"], dtype=np.float32)  # [n_chunks,128,CHUNK_W,C]
        by_pos = r.transpose(0, 2, 1, 3).reshape(N_WIN, W, C)
        core = out[k * RNG : (k + 1) * RNG].reshape(N_WIN, W, C)
        core[perm[k]] = by_pos
    return out[:n_coarse]


def _install_axon_hooks_shim():
    """Provide antenv.axon_hooks + the ctypes NTFF hook if the image lacks it."""
    import contextlib
    import ctypes
    import types

    try:
        from antenv.axon_hooks import get_axon_ntff_profile_hook  # noqa: F401

        return
    except ImportError:
        pass
    import antenv

    mod = types.ModuleType("antenv.axon_hooks")
    state = {"h": None}
    mod.set_axon_ntff_profile_hook = lambda h: state.__setitem__("h", h)
    mod.get_axon_ntff_profile_hook = lambda: state["h"]
    antenv.axon_hooks = mod
    sys.modules["antenv.axon_hooks"] = mod

    so_path = "/opt/axon/libaxon_pjrt.so"
    if not os.path.exists(so_path):
        return
    lib = ctypes.CDLL(so_path)
    if not hasattr(lib, "axon_start_nrt_profile"):
        return
    lib.axon_start_nrt_profile.argtypes = [
        ctypes.POINTER(ctypes.c_int64),
        ctypes.c_size_t,
    ]
    lib.axon_start_nrt_profile.restype = ctypes.c_int64
    lib.axon_stop_nrt_profile.argtypes = [ctypes.c_char_p]
    lib.axon_stop_nrt_profile.restype = ctypes.c_int64

    @contextlib.contextmanager
    def _hook(output_dir, device_ids):
        import jax

        jax.devices()
        if device_ids:
            idsv = (ctypes.c_int64 * len(device_ids))(*device_ids)
            rc = lib.axon_start_nrt_profile(idsv, len(device_ids))
        else:
            rc = lib.axon_start_nrt_profile(None, 0)
        if rc != 0:
            raise RuntimeError(f"axon_start_nrt_profile rc={rc}")
        try:
            yield
        finally:
            nfiles = lib.axon_stop_nrt_profile(str(output_dir).encode())
            print(f"profile: {nfiles} file(s) written to {output_dir}", file=sys.stderr)

    state["h"] = _hook


def _per_core_counts(coarse_ids):
    """[NCORES][N_WIN][W] token counts."""
    ids = np.asarray(coarse_ids, dtype=np.int64).ravel()
    cnt = np.bincount(ids, minlength=N_COARSE_PAD)
    return cnt.reshape(NCORES, N_WIN, W)


def kernel(fine_feats, coarse_ids, num_coarse):
    global LAST_RESULT
    from concourse.bass_utils import run_bass_kernel_spmd

    counts = _per_core_counts(coarse_ids)
    structure, cells, perm = build_structure(counts)
    layout = Layout(structure)
    in_maps = shard_inputs(fine_feats, coarse_ids, layout, cells, perm)
    key = tuple(tuple(s) for s in structure)
    if key not in _nc_cache:
        _nc_cache.clear()
        _nc_cache[key] = build_nc(layout)
    nc = _nc_cache[key]
    trace = bool(int(os.environ.get("KERNEL_TRACE", "0")))
    if trace:
        _install_axon_hooks_shim()
    res = run_bass_kernel_spmd(nc, in_maps, core_ids=list(range(NCORES)), trace=trace)
    LAST_RESULT = res
    return assemble_output(res.results, int(num_coarse), perm)
